# revision 3
# baseline (speedup 1.0000x reference)
"""CorrelateAttention Trainium2 kernel.

Computes, for hidden_states [B=4, L=2048, C=2048]:
    qk = hidden @ W.T + b            -> split into 16 q heads / 4 kv heads (GQA, d=128)
    q scaled per-dim by softplus-derived scale (folded into W on host)
    logits = q @ k.T / sqrt(128)     (sqrt folded into W on host)
    out = mean_h softmax(logits)     -> [B, 2048, 2048]

Sharding: 8 cores = 4 batches x 2 head-halves. Each core computes
sum_{h in its 8 heads} softmax_h for its batch -> [2048, 2048] partial.
Host combines: out[b] = (core[2b] + core[2b+1]) / 16.

Per-core kernel phases (all matmuls bf16, fp32 PSUM accumulation):
  1) proj: QT/KT[d=128, L] per head block, contraction over hidden dim in
     PSUM, bias fused into the PSUM->SBUF cast on ScalarE.
  2) attention: per (q-block, head): logits PSUM tile [128, 2048]; exp +
     row-sum in one ScalarE activation; fast reciprocal on DVE; fused
     acc += exp * (1/sum) in one DVE scalar_tensor_tensor pass.
"""

import math
import os

import numpy as np

from concourse import bass, bacc, mybir, tile
from concourse.bass_utils import run_bass_kernel_spmd

B = 4
L = 2048
C = 2048
HEAD_DIM = 128
NUM_HEADS = 16
NUM_K_HEADS = 4
R_SOFTPLUS_0 = 1.442695041

N_CORES = 8
NH = 8          # q heads per core
NG = 2          # kv heads per core
NDBLK = NH + NG  # 10 projection 128-row blocks per core
NCT = C // 128   # 16 contraction tiles
NQB = L // 128   # 16 query blocks

F32 = mybir.dt.float32
BF16 = mybir.dt.bfloat16

# matmul moving-operand chunk (bf16 allows 1024)
MM_N = int(os.environ.get("CORR_MM_N", "512"))


def _kernel_body(tc, out_dram, hT, wT, bias):
    nc = tc.nc
    nch = L // MM_N

    with tc.tile_pool(name="persist", bufs=1) as persist, \
         tc.tile_pool(name="psum", bufs=2, space="PSUM") as psum_pool:

        bias_t = persist.tile([128, NDBLK], F32, tag="bias", name="bias_t")
        nc.sync.dma_start(bias_t[:], bias[:])

        # persistent QT/KT tiles: [128 d, L] per head block
        qkt = []
        for db in range(NDBLK):
            t = persist.tile([128, L], BF16, tag=f"qkt{db}", name=f"qkt{db}")
            qkt.append(t)

        # ---- Phase 1: projection ----
        with tc.tile_pool(name="hpool", bufs=1) as hpool, \
             tc.tile_pool(name="wpool", bufs=1) as wpool:
            h_tiles = []
            for c in range(NCT):
                ht = hpool.tile([128, L], BF16, tag=f"h{c}", name=f"h{c}")
                nc.sync.dma_start(ht[:], hT[c * 128:(c + 1) * 128, :])
                h_tiles.append(ht)
            w_tiles = []
            for db in range(NDBLK):
                wt = wpool.tile([128, NCT * 128], BF16, tag=f"w{db}", name=f"w{db}")
                nc.sync.dma_start(wt[:], wT[db])
                w_tiles.append(wt)

            for db in range(NDBLK):
                pt = psum_pool.tile([128, L], F32, tag="psum", name=f"proj_ps{db}")
                for c in range(NCT):
                    for j in range(nch):
                        nc.tensor.matmul(
                            pt[:, j * MM_N:(j + 1) * MM_N],
                            w_tiles[db][:, c * 128:(c + 1) * 128],
                            h_tiles[c][:, j * MM_N:(j + 1) * MM_N],
                            start=(c == 0),
                            stop=(c == NCT - 1),
                        )
                # PSUM -> SBUF bf16 cast with fused bias add
                nc.scalar.activation(
                    qkt[db][:],
                    pt[:],
                    mybir.ActivationFunctionType.Identity,
                    bias=bias_t[:, db:db + 1],
                )

        # ---- Phase 2: attention scores ----
        with tc.tile_pool(name="expp", bufs=3) as expp, \
             tc.tile_pool(name="accp", bufs=2) as accp, \
             tc.tile_pool(name="smallp", bufs=8) as smallp:
            for qb in range(NQB):
                acc = accp.tile([128, L], F32, tag="acc", name=f"acc{qb}")
                for h in range(NH):
                    g = NH + h // 4  # kv block index in qkt
                    pt = psum_pool.tile([128, L], F32, tag="psum", name=f"att_ps{qb}_{h}")
                    for j in range(nch):
                        nc.tensor.matmul(
                            pt[:, j * MM_N:(j + 1) * MM_N],
                            qkt[h][:, qb * 128:(qb + 1) * 128],
                            qkt[g][:, j * MM_N:(j + 1) * MM_N],
                            start=True,
                            stop=True,
                        )
                    exp_t = expp.tile([128, L], F32, tag="exp", name=f"exp{qb}_{h}")
                    sum_t = smallp.tile([128, 1], F32, tag="sum", name=f"sum{qb}_{h}")
                    nc.scalar.activation(
                        exp_t[:],
                        pt[:],
                        mybir.ActivationFunctionType.Exp,
                        accum_out=sum_t[:],
                    )
                    r_t = smallp.tile([128, 1], F32, tag="r", name=f"r{qb}_{h}")
                    nc.vector.reciprocal_approx_fast(r_t[:], sum_t[:])
                    if h == 0:
                        nc.vector.tensor_scalar_mul(acc[:], exp_t[:], r_t[:])
                    else:
                        nc.vector.scalar_tensor_tensor(
                            out=acc[:],
                            in0=exp_t[:],
                            scalar=r_t[:],
                            in1=acc[:],
                            op0=mybir.AluOpType.mult,
                            op1=mybir.AluOpType.add,
                        )
                nc.sync.dma_start(out_dram[qb * 128:(qb + 1) * 128, :], acc[:])


_PROGRAM = None


def _build_program():
    global _PROGRAM
    if _PROGRAM is not None:
        return _PROGRAM
    nc = bacc.Bacc(
        "TRN2",
        target_bir_lowering=False,
        debug=False,
        num_devices=N_CORES,
    )
    hT = nc.dram_tensor("hT", [C, L], BF16, kind="ExternalInput").ap()
    # wT pre-swizzled on host into SBUF tile layout:
    # wT[db, p, c_hi*128 + d] = W_block[db][c_hi*128 + p, d]
    wT = nc.dram_tensor("wT", [NDBLK, 128, NCT * 128], BF16, kind="ExternalInput").ap()
    bias = nc.dram_tensor("bias", [128, NDBLK], F32, kind="ExternalInput").ap()
    out = nc.dram_tensor("out", [L, L], F32, kind="ExternalOutput").ap()
    with tile.TileContext(nc) as tc:
        _kernel_body(tc, out, hT, wT, bias)
    nc.compile()
    _PROGRAM = nc
    return nc


def _prep_core_inputs(hidden_states, qk_weight, qk_bias, scaling):
    """Host-side fold + shard. Returns list of 8 in_maps."""
    np_bf16 = mybir.dt.np(BF16)

    Q_SIZE = NUM_HEADS * HEAD_DIM
    # per-dim q scale, with the extra 1/sqrt(d) logits scale folded in
    sp = np.logaddexp(0.0, scaling.astype(np.float64))  # softplus
    qscale = (R_SOFTPLUS_0 / math.sqrt(HEAD_DIM)) * sp / math.sqrt(HEAD_DIM)

    W = qk_weight.astype(np.float64)
    bvec = qk_bias.astype(np.float64)
    Wq = W[:Q_SIZE].reshape(NUM_HEADS, HEAD_DIM, C) * qscale[None, :, None]
    bq = bvec[:Q_SIZE].reshape(NUM_HEADS, HEAD_DIM) * qscale[None, :]
    Wk = W[Q_SIZE:].reshape(NUM_K_HEADS, HEAD_DIM, C)
    bk = bvec[Q_SIZE:].reshape(NUM_K_HEADS, HEAD_DIM)

    in_maps = []
    for core in range(N_CORES):
        b = core // 2
        half = core % 2
        heads = slice(half * NH, half * NH + NH)
        kvs = slice(half * NG, half * NG + NG)
        # [NDBLK, 128 d, C] row blocks: 8 q heads then 2 kv heads
        w_blocks = np.concatenate([Wq[heads], Wk[kvs]], axis=0)
        # swizzle into SBUF tile layout [NDBLK, 128 p, NCT*128]:
        # wT[db, p, c_hi*128 + d] = w_blocks[db, d, c_hi*128 + p]
        wsw = w_blocks.reshape(NDBLK, HEAD_DIM, NCT, 128).transpose(0, 3, 2, 1)
        wT_core = np.ascontiguousarray(wsw.reshape(NDBLK, 128, NCT * 128)).astype(np_bf16)
        bias_core = np.ascontiguousarray(
            np.concatenate([bq[heads], bk[kvs]], axis=0).T).astype(np.float32)
        hT_core = np.ascontiguousarray(hidden_states[b].T).astype(np_bf16)
        in_maps.append({"hT": hT_core, "wT": wT_core, "bias": bias_core})
    return in_maps


def kernel(hidden_states, qk_weight, qk_bias, scaling, **run_kwargs):
    nc = _build_program()
    in_maps = _prep_core_inputs(hidden_states, qk_weight, qk_bias, scaling)
    res = run_bass_kernel_spmd(nc, in_maps, list(range(N_CORES)), **run_kwargs)
    out = np.empty((B, L, L), dtype=np.float32)
    for b in range(B):
        out[b] = (res.results[2 * b]["out"] + res.results[2 * b + 1]["out"]) / NUM_HEADS
    if run_kwargs:
        kernel.last_result = res
    return out


# revision 22
# speedup vs baseline: 234.7159x; 234.7159x over previous
"""CorrelateAttention Trainium2 kernel.

Computes, for hidden_states [B=4, L=2048, C=2048]:
    qk = hidden @ W.T + b            -> split into 16 q heads / 4 kv heads (GQA, d=128)
    q scaled per-dim by softplus-derived scale (folded into W on host)
    logits = q @ k.T / sqrt(128)     (sqrt folded into W on host)
    out = mean_h softmax(logits)     -> [B, 2048, 2048]

Sharding: 8 cores = 4 batches x 2 head-halves. Each core computes
sum_{h in its 8 heads} softmax_h for its batch -> [2048, 2048] partial.
Host combines: out[b] = (core[2b] + core[2b+1]) / 16.

Per-core kernel (all matmuls bf16, fp32 PSUM accumulation):
  - proj: QT/KT[d=128, L] per head block; KV blocks first, then q-head
    blocks in head order so attention for head h can start right after
    its projection lands.
  - attention rows interleaved with proj: for each head h, all q-blocks:
    logits PSUM tile [128, 2048]; exp + row-sum in one ScalarE activation;
    reciprocal on DVE; fused acc[qb] += exp * (1/sum) in one DVE
    scalar_tensor_tensor pass.
"""

import math
import os

import numpy as np

from concourse import bacc, mybir, tile
from concourse.bass_utils import run_bass_kernel_spmd

B = 4
L = 2048
C = 2048
HEAD_DIM = 128
NUM_HEADS = 16
NUM_K_HEADS = 4
R_SOFTPLUS_0 = 1.442695041

N_CORES = 8
NH = 8          # q heads per core
NG = 2          # kv heads per core
NDBLK = NH + NG  # 10 projection 128-row blocks per core
NCT = C // 128   # 16 contraction tiles
NQB = L // 128   # 16 query blocks

F32 = mybir.dt.float32
BF16 = mybir.dt.bfloat16

MM_N = int(os.environ.get("CORR_MM_N", "512"))     # matmul moving chunk
SOFTMAX_BF16 = os.environ.get("CORR_SOFTMAX", "f32") == "bf16"
# engine for the exp*r scale mults: pool | dve | stt (fused, DVE 1x)
MUL_ENGINE = os.environ.get("CORR_MUL_ENGINE", "stt")
# engine for the proj PSUM->SBUF bias/cast copies: act | dve
COPY_ENGINE = os.environ.get("CORR_COPY_ENGINE", "act")
# q-block group sizes (acc tiles resident per group)
_groups_env = os.environ.get("CORR_QB_GROUPS")
if _groups_env:
    QB_GROUPS = tuple(int(x) for x in _groups_env.split(","))
else:
    QB_GROUPS = (16,) if SOFTMAX_BF16 else (8, 8)
assert sum(QB_GROUPS) == NQB


def _proj_block(nc, psum_pool, w_pool, wT, h_tiles, qkt, bias_t, db, wt=None):
    nch = L // MM_N
    if wt is None:
        wt = w_pool.tile([128, NCT * 128], BF16, tag="w", name=f"w{db}")
        nc.sync.dma_start(wt[:], wT[db])
    pt = psum_pool.tile([128, L], F32, tag="psum", name=f"proj_ps{db}")
    for c in range(NCT):
        for j in range(nch):
            nc.tensor.matmul(
                pt[:, j * MM_N:(j + 1) * MM_N],
                wt[:, c * 128:(c + 1) * 128],
                h_tiles[c][:, j * MM_N:(j + 1) * MM_N],
                start=(c == 0),
                stop=(c == NCT - 1),
            )
    # PSUM -> SBUF bf16 cast with fused bias add
    if COPY_ENGINE == "dve":
        nc.vector.tensor_scalar_add(qkt[db][:], pt[:], bias_t[:, db:db + 1])
    else:
        nc.scalar.activation(
            qkt[db][:],
            pt[:],
            mybir.ActivationFunctionType.Identity,
            bias=bias_t[:, db:db + 1],
        )


def _attn_row(nc, psum_pool, expp, smallp, qkt, acc_tiles, out_dram, h, qbs):
    """Attention for head h over the q-blocks in `qbs`."""
    nch = L // MM_N
    sm_dt = BF16 if SOFTMAX_BF16 else F32
    g = NH + h // 4  # kv block index in qkt
    for qb in qbs:
        pt = psum_pool.tile([128, L], F32, tag="psum", name=f"att_ps{qb}_{h}")
        for j in range(nch):
            nc.tensor.matmul(
                pt[:, j * MM_N:(j + 1) * MM_N],
                qkt[h][:, qb * 128:(qb + 1) * 128],
                qkt[g][:, j * MM_N:(j + 1) * MM_N],
                start=True,
                stop=True,
            )
        exp_t = expp.tile([128, L], sm_dt, tag="exp", name=f"exp{qb}_{h}")
        sum_t = smallp.tile([128, 1], F32, tag="sum", name=f"sum{qb}_{h}")
        nc.scalar.activation(
            exp_t[:],
            pt[:],
            mybir.ActivationFunctionType.Exp,
            accum_out=sum_t[:],
        )
        r_t = smallp.tile([128, 1], F32, tag="r", name=f"r{qb}_{h}")
        nc.vector.reciprocal(r_t[:], sum_t[:])
        acc = acc_tiles[qb]
        if h == 0:
            nc.vector.tensor_scalar_mul(acc[:], exp_t[:], r_t[:])
        elif MUL_ENGINE == "stt" or (MUL_ENGINE == "mix" and h % 4 != 3):
            nc.vector.scalar_tensor_tensor(
                out=acc[:],
                in0=exp_t[:],
                scalar=r_t[:],
                in1=acc[:],
                op0=mybir.AluOpType.mult,
                op1=mybir.AluOpType.add,
            )
        elif MUL_ENGINE == "mix":
            nc.gpsimd.scalar_tensor_tensor(
                out=acc[:],
                in0=exp_t[:],
                scalar=r_t[:],
                in1=acc[:],
                op0=mybir.AluOpType.mult,
                op1=mybir.AluOpType.add,
            )
        else:
            eng = nc.gpsimd if MUL_ENGINE == "pool" else nc.vector
            tmp = expp.tile([128, L], sm_dt, tag="tmp", name=f"tmp{qb}_{h}")
            eng.tensor_scalar_mul(tmp[:], exp_t[:], r_t[:])
            nc.vector.tensor_tensor(
                out=acc[:], in0=acc[:], in1=tmp[:], op=mybir.AluOpType.add)
        if h == NH - 1:
            nc.sync.dma_start(out_dram[qb * 128:(qb + 1) * 128, :], acc[:])


def _kernel_body(tc, out_dram, hT, wT, bias):
    nc = tc.nc
    sm_dt = BF16 if SOFTMAX_BF16 else F32

    with tc.tile_pool(name="persist", bufs=1) as persist, \
         tc.tile_pool(name="psum", bufs=2, space="PSUM") as psum_pool, \
         tc.tile_pool(name="expp", bufs=3) as expp, \
         tc.tile_pool(name="smallp", bufs=8) as smallp:

        bias_t = persist.tile([128, NDBLK], F32, tag="bias", name="bias_t")
        nc.sync.dma_start(bias_t[:], bias[:])

        qkt = [persist.tile([128, L], BF16, tag=f"qkt{db}", name=f"qkt{db}")
               for db in range(NDBLK)]

        first_grp = QB_GROUPS[0]
        with tc.tile_pool(name="accpA", bufs=1) as accpA:
            qbsA = list(range(first_grp))
            accA = {qb: accpA.tile([128, L], sm_dt, tag=f"acc{qb}", name=f"acc{qb}")
                    for qb in qbsA}

            # h/w pools sit on top of the pool stack and are released as soon
            # as the last projection is emitted, so later acc groups reuse
            # their SBUF region
            hpool = tc.alloc_tile_pool(name="hpool", bufs=1)
            w_pool = tc.alloc_tile_pool(name="wpool", bufs=2)
            # prefetch the first two weight blocks ahead of the h stream so
            # the first projections overlap the h DMAs
            wt_first = []
            for db in (NH, 0):
                wt = w_pool.tile([128, NCT * 128], BF16, tag="w", name=f"w{db}")
                nc.sync.dma_start(wt[:], wT[db])
                wt_first.append(wt)
            h_tiles = []
            for c in range(NCT):
                ht = hpool.tile([128, L], BF16, tag=f"h{c}", name=f"h{c}")
                nc.sync.dma_start(ht[:], hT[c * 128:(c + 1) * 128, :])
                h_tiles.append(ht)

            # first kv block + first q head, so attention row 0 starts early
            _proj_block(nc, psum_pool, w_pool, wT, h_tiles, qkt, bias_t, NH,
                        wt=wt_first[0])
            _proj_block(nc, psum_pool, w_pool, wT, h_tiles, qkt, bias_t, 0,
                        wt=wt_first[1])

            # interleave: attention row h, then the next projection(s);
            # kv block 9 must land before row 4 needs it. Wide first groups
            # (lots of ACT work per row) take the projections packed densely;
            # narrow ones spread them to avoid starving ScalarE.
            if first_grp >= 10:
                proj_after = [[1], [2, 3], [NH + 1, 4], [5, 6], [7], [], [], []]
                release_after = 4
            else:
                proj_after = [[1], [2, NH + 1], [3], [4], [5], [6], [7], []]
                release_after = 6
            for h in range(NH):
                _attn_row(nc, psum_pool, expp, smallp, qkt, accA, out_dram, h, qbsA)
                for db in proj_after[h]:
                    _proj_block(nc, psum_pool, w_pool, wT, h_tiles, qkt, bias_t, db)
                if h == release_after:
                    w_pool.release()
                    hpool.release()

        qb_start = first_grp
        for grp in QB_GROUPS[1:]:
            qbs = list(range(qb_start, qb_start + grp))
            qb_start += grp
            with tc.tile_pool(name=f"accp{qbs[0]}", bufs=1) as accp:
                acc = {qb: accp.tile([128, L], sm_dt, tag=f"acc{qb}", name=f"acc{qb}")
                       for qb in qbs}
                for h in range(NH):
                    _attn_row(nc, psum_pool, expp, smallp, qkt, acc, out_dram, h, qbs)


_PROGRAM = None


def _build_program():
    global _PROGRAM
    if _PROGRAM is not None:
        return _PROGRAM
    nc = bacc.Bacc(
        "TRN2",
        target_bir_lowering=False,
        debug=False,
        num_devices=N_CORES,
    )
    out_dt = BF16 if SOFTMAX_BF16 else F32
    hT = nc.dram_tensor("hT", [C, L], BF16, kind="ExternalInput").ap()
    # wT pre-swizzled on host into SBUF tile layout:
    # wT[db, p, c_hi*128 + d] = W_block[db][c_hi*128 + p, d]
    wT = nc.dram_tensor("wT", [NDBLK, 128, NCT * 128], BF16, kind="ExternalInput").ap()
    bias = nc.dram_tensor("bias", [128, NDBLK], F32, kind="ExternalInput").ap()
    out = nc.dram_tensor("out", [L, L], out_dt, kind="ExternalOutput").ap()
    with tile.TileContext(nc) as tc:
        _kernel_body(tc, out, hT, wT, bias)
    nc.compile()
    _PROGRAM = nc
    return nc


def _prep_core_inputs(hidden_states, qk_weight, qk_bias, scaling):
    """Host-side fold + shard. Returns list of 8 in_maps."""
    np_bf16 = mybir.dt.np(BF16)

    Q_SIZE = NUM_HEADS * HEAD_DIM
    # per-dim q scale, with the extra 1/sqrt(d) logits scale folded in
    sp = np.logaddexp(0.0, scaling.astype(np.float64))  # softplus
    qscale = (R_SOFTPLUS_0 / math.sqrt(HEAD_DIM)) * sp / math.sqrt(HEAD_DIM)

    W = qk_weight.astype(np.float64)
    bvec = qk_bias.astype(np.float64)
    Wq = W[:Q_SIZE].reshape(NUM_HEADS, HEAD_DIM, C) * qscale[None, :, None]
    bq = bvec[:Q_SIZE].reshape(NUM_HEADS, HEAD_DIM) * qscale[None, :]
    Wk = W[Q_SIZE:].reshape(NUM_K_HEADS, HEAD_DIM, C)
    bk = bvec[Q_SIZE:].reshape(NUM_K_HEADS, HEAD_DIM)

    in_maps = []
    for core in range(N_CORES):
        b = core // 2
        half = core % 2
        heads = slice(half * NH, half * NH + NH)
        kvs = slice(half * NG, half * NG + NG)
        # [NDBLK, 128 d, C] row blocks: 8 q heads then 2 kv heads
        w_blocks = np.concatenate([Wq[heads], Wk[kvs]], axis=0)
        # swizzle into SBUF tile layout [NDBLK, 128 p, NCT*128]:
        # wT[db, p, c_hi*128 + d] = w_blocks[db, d, c_hi*128 + p]
        wsw = w_blocks.reshape(NDBLK, HEAD_DIM, NCT, 128).transpose(0, 3, 2, 1)
        wT_core = np.ascontiguousarray(wsw.reshape(NDBLK, 128, NCT * 128)).astype(np_bf16)
        bias_core = np.ascontiguousarray(
            np.concatenate([bq[heads], bk[kvs]], axis=0).T).astype(np.float32)
        hT_core = np.ascontiguousarray(hidden_states[b].T).astype(np_bf16)
        in_maps.append({"hT": hT_core, "wT": wT_core, "bias": bias_core})
    return in_maps


def kernel(hidden_states, qk_weight, qk_bias, scaling):
    nc = _build_program()
    in_maps = _prep_core_inputs(
        np.asarray(hidden_states), np.asarray(qk_weight),
        np.asarray(qk_bias), np.asarray(scaling))
    res = run_bass_kernel_spmd(nc, in_maps, list(range(N_CORES)))
    out = np.empty((B, L, L), dtype=np.float32)
    for b in range(B):
        out[b] = (res.results[2 * b]["out"].astype(np.float32)
                  + res.results[2 * b + 1]["out"].astype(np.float32)) / NUM_HEADS
    return out
